# revision 8
# baseline (speedup 1.0000x reference)
"""Trainium2 Bass kernel for a top-k BCE + soft-Dice loss.

Math
----
reference computes, over n = 9,437,184 elements:
  bce_map = softplus(x) - x*t          (elementwise, stable BCE-with-logits)
  bce     = mean(top_k(bce_map, k)),   k = int(0.2 * n)
  p       = sigmoid(x)
  dice    = (2*sum(p*t) + eps) / (sum(p) + sum(t) + eps)
  loss    = bce + 0.5*(1 - dice)

Key identity: for tau* = k-th largest of bce_map,
  sum_topk = k*tau* + sum(relu(bce_map - tau*))        (exact)
and the RHS is *second-order* insensitive to errors in tau (derivative is
k - count(bce > tau) = 0 at tau*).  So a host-side subsample estimate of tau
(error ~1e-3 -> relative loss error ~1e-6) lets the device compute the whole
loss in a single streaming pass over the inputs — no distributed top-k.

Device pass (data-parallel over 8 cores, shard = contiguous 1/8 of the flat
arrays viewed as [128 partitions x 9216 cols], tiles of 1536 cols):
  ACT : e = exp(x); sp = ln(e+1); em = exp(-sp) (= 1-sigmoid(x)) with
        fused accumulation -> sum(em)
  DVE : xt = x*t; bce = sp - xt;
        tensor_scalar (bce - tau) max 0 with accumulation -> sum(relu)
        tensor_tensor_reduce em*t -> sum(em*t)
  PE  : ones[128,1]^T @ t -> per-column partial sums -> sum(t)
Host merges the tiny per-core partials in float64:
  sum(p) = n - sum(em),  sum(p*t) = sum(t) - sum(em*t).
"""

import os

import numpy as np

N_CORES = 8
P = 128
FD = 1536               # columns per compute tile
NT = 6                  # tiles per core
COLS = FD * NT          # 9216 columns per core
SHARD = P * COLS        # 1,179,648 elements per core
N_TOTAL = N_CORES * SHARD
TOPK_RATIO = 0.2
DICE_WEIGHT = 0.5
DICE_EPS = 1e-6

_BUILT = {}
LAST_RESULTS = None     # BassKernelResults of the most recent device run


def _build():
    """Trace the Bass/Tile program once; reuse across calls."""
    if "nc" in _BUILT:
        return _BUILT["nc"]

    import concourse.tile as tile
    from concourse import bacc, mybir

    dt = mybir.dt.float32
    Alu = mybir.AluOpType
    Act = mybir.ActivationFunctionType

    nc = bacc.Bacc("TRN2", target_bir_lowering=False, debug=False)
    xl = nc.dram_tensor("xl", [P, COLS], dt, kind="ExternalInput")
    tg = nc.dram_tensor("tg", [P, COLS], dt, kind="ExternalInput")
    taub = nc.dram_tensor("taub", [P, 1], dt, kind="ExternalInput")
    # stats cols: [0:NT) sum(em) | [NT:2NT) sum(relu) | [2NT:3NT) sum(em*t)
    # | [3NT:4NT) sum(t)
    stats = nc.dram_tensor("stats", [P, 4 * NT], dt, kind="ExternalOutput")

    with tile.TileContext(nc) as tc:
        with (
            tc.tile_pool(name="io", bufs=3) as io,
            tc.tile_pool(name="mid", bufs=2) as mid,
            tc.tile_pool(name="small", bufs=1) as small,
        ):
            tau_sb = small.tile([P, 1], dt)
            nc.sync.dma_start(out=tau_sb[:], in_=taub.ap())
            stats_sb = small.tile([P, 4 * NT], dt)

            for i in range(NT):
                x = io.tile([P, FD], dt, tag="x")
                t = io.tile([P, FD], dt, tag="t")
                nc.sync.dma_start(out=x[:], in_=xl.ap()[:, i * FD:(i + 1) * FD])
                nc.sync.dma_start(out=t[:], in_=tg.ap()[:, i * FD:(i + 1) * FD])

                e = mid.tile([P, FD], dt, tag="e")
                nc.scalar.activation(e[:], x[:], Act.Exp)
                sp = mid.tile([P, FD], dt, tag="sp")
                nc.scalar.activation(sp[:], e[:], Act.Ln, bias=1.0)
                em = mid.tile([P, FD], dt, tag="em")
                nc.scalar.activation(
                    em[:], sp[:], Act.Exp, scale=-1.0,
                    accum_out=stats_sb[:, i:i + 1],
                )

                xt = mid.tile([P, FD], dt, tag="xt")
                nc.gpsimd.tensor_mul(xt[:], x[:], t[:])
                bce = mid.tile([P, FD], dt, tag="bce")
                nc.vector.tensor_sub(bce[:], sp[:], xt[:])
                # accum = sum(max(bce, tau)); host subtracts n*tau to get
                # sum(relu(bce - tau))
                rl = mid.tile([P, FD], dt, tag="rl")
                nc.vector.tensor_scalar(
                    rl[:], bce[:], tau_sb[:], None,
                    op0=Alu.max, op1=Alu.add,
                    accum_out=stats_sb[:, NT + i:NT + i + 1],
                )
                emt = mid.tile([P, FD], dt, tag="emt")
                nc.vector.scalar_tensor_tensor(
                    emt[:], em[:], 1.0, t[:],
                    op0=Alu.mult, op1=Alu.mult,
                    accum_out=stats_sb[:, 2 * NT + i:2 * NT + i + 1],
                )
                tsum = mid.tile([P, FD], dt, tag="tsum")
                nc.vector.tensor_scalar(
                    tsum[:], t[:], 1.0, None,
                    op0=Alu.mult, op1=Alu.add,
                    accum_out=stats_sb[:, 3 * NT + i:3 * NT + i + 1],
                )

            nc.sync.dma_start(out=stats.ap(), in_=stats_sb[:])

    nc.compile()
    _BUILT["nc"] = nc
    return nc


def _estimate_tau(xf, tf, k, n):
    """k-th largest of the BCE map, estimated from a strided subsample."""
    xs = xf[::7].astype(np.float64)
    ts = tf[::7].astype(np.float64)
    b = np.maximum(xs, 0.0) - xs * ts + np.log1p(np.exp(-np.abs(xs)))
    m = b.size
    kk = max(1, min(m, int(round(m * (k / n)))))
    return float(np.partition(b, m - kk)[m - kk])


def kernel(logits: np.ndarray, targets: np.ndarray) -> np.ndarray:
    global LAST_RESULTS
    from concourse import bass_utils

    xf = np.ascontiguousarray(logits, dtype=np.float32).reshape(-1)
    tf = np.ascontiguousarray(targets, dtype=np.float32).reshape(-1)
    n = xf.size
    assert n == N_TOTAL, f"kernel hardcoded for {N_TOTAL} elements, got {n}"
    k = max(1, int(n * TOPK_RATIO))

    tau = _estimate_tau(xf, tf, k, n)
    taub = np.full((P, 1), tau, dtype=np.float32)

    xs = xf.reshape(N_CORES, P, COLS)
    ts = tf.reshape(N_CORES, P, COLS)
    in_maps = [
        {"xl": xs[c], "tg": ts[c], "taub": taub}
        for c in range(N_CORES)
    ]

    nc = _build()
    trace = os.environ.get("KERNEL_TRACE", "0") == "1"
    res = bass_utils.run_bass_kernel_spmd(
        nc, in_maps, core_ids=list(range(N_CORES)), trace=trace,
    )
    LAST_RESULTS = res

    sum_em = 0.0
    sum_max = 0.0
    sum_emt = 0.0
    sum_t = 0.0
    for r in res.results:
        st = r["stats"].astype(np.float64)
        sum_em += st[:, 0:NT].sum()
        sum_max += st[:, NT:2 * NT].sum()
        sum_emt += st[:, 2 * NT:3 * NT].sum()
        sum_t += st[:, 3 * NT:4 * NT].sum()

    sum_rl = sum_max - n * tau      # sum(max(bce,tau)) - n*tau = sum(relu)
    sum_topk = k * tau + sum_rl
    bce_mean = sum_topk / k
    sum_p = n - sum_em
    sum_pt = sum_t - sum_emt
    dice = (2.0 * sum_pt + DICE_EPS) / (sum_p + sum_t + DICE_EPS)
    loss = bce_mean + DICE_WEIGHT * (1.0 - dice)
    return np.array(loss, dtype=np.float32)
